# revision 55
# baseline (speedup 1.0000x reference)
"""Multi-head attention (B=4, S=2048, D=1024, H=16) on 8 TRN2 NeuronCores.

Sharding: core c -> (batch b = c//2, head-group g = c%2 of 8 heads).
Data parallel over batch, tensor parallel over heads; each core computes
its group's QKV projection slices, causal attention for its 8 heads, and
the partial output projection. Host sums the two per-batch partials
(the tensor-parallel unshard) and adds the V-bias epilogue.

On-device layout is "features on partitions": x, Q, K arrive/stay
transposed [feat, seq]; attention scores are computed directly in
transposed form S.T[k, q] so the exp'd probabilities feed the PV matmul
without any on-chip transpose. The softmax denominator rides the PV
matmul as an appended ones-column of V; normalization is a fast
reciprocal + K=1 broadcast matmul + DVE multiply. Causal masking uses
suffix-restricted tiles plus a small multiplicative 0/1 strip on the
exp'd probabilities; non-staircase masks fall back to additive -1e9
biases on the scores.

The attention inner loop is software-pipelined to keep the PE array
busy (and HAM unthrottled): PV matmuls lag the score matmuls by one
k-tile, and the per-head-pair normalization + per-q-tile output
projection are deferred into the next head-pair's score stream.
"""

import os
import numpy as np

B, S, D, H = 4, 2048, 1024, 16
DK = D // H          # 64
HPC = H // 2         # heads per core = 8
GD = HPC * DK        # group feature width = 512
QT = 512             # q-tile width (free dim of S.T chunks)
KTL = 128            # k-tile length (partition dim of S.T chunks)
N_QT = S // QT       # 4
N_KT = S // KTL      # 16
SB = 512             # phase-1 seq block
STRW = 128           # width of multiplicative mask strips
NEG = np.float32(-1e9)
SCALE = 1.0 / np.sqrt(np.float32(DK))

_cache = {}
last_results = None


def _classify_mask(mask2d):
    """Classify each (q-tile, k-tile) block of the [S,S] bool mask.

    Returns (plan, strips, biases):
      plan[qi] = list over valid kt of (kt, kind, a, b):
        kind 0 = clean (no masking)
        kind 1 = staircase: a = q0 (suffix start), b = (strip_idx, strip_w)
        kind 2 = general:   a = bias_idx
      strips: list of [KTL, STRW] f32 0/1 multiplicative masks
      biases: list of [KTL, QT] f32 additive -1e9/0 masks
    Blocks are in S.T (k, q) layout.
    """
    kl = np.arange(KTL)[:, None]
    ql = np.arange(QT)[None, :]
    plan = []
    strips, strip_keys = [], {}
    biases, bias_keys = [], {}
    for qi in range(N_QT):
        row = []
        for kt in range(N_KT):
            blk = mask2d[qi * QT:(qi + 1) * QT, kt * KTL:(kt + 1) * KTL].T
            if blk.all():
                continue
            if not blk.any():
                row.append((kt, 0, 0, None))
                continue
            dj = kt * KTL - qi * QT
            stair = (0 <= dj <= QT - KTL and np.array_equal(blk, kl + dj > ql)
                     and not os.environ.get("KERNEL_NO_STAIR"))
            if stair and (dj == 0 or row):
                # masked region spans exactly q in [dj, dj+KTL)
                w = min(dj + KTL, QT) - dj
                pat = (~blk[:, dj:dj + w]).astype(np.float32)
                key = (w, pat.tobytes())
                if key not in strip_keys:
                    strip_keys[key] = len(strips)
                    p = np.zeros((KTL, STRW), np.float32)
                    p[:, :w] = pat
                    strips.append(p)
                row.append((kt, 1, dj, (strip_keys[key], w)))
            else:
                bias = np.where(blk, NEG, np.float32(0.0))
                key = bias.tobytes()
                if key not in bias_keys:
                    bias_keys[key] = len(biases)
                    biases.append(bias)
                row.append((kt, 2, bias_keys[key], None))
        if not row:
            # fully-masked q-row: include everything with full bias so the
            # softmax matches the reference's uniform distribution.
            bias = np.full((KTL, QT), NEG, np.float32)
            key = bias.tobytes()
            if key not in bias_keys:
                bias_keys[key] = len(biases)
                biases.append(bias)
            row = [(kt, 2, bias_keys[key], None) for kt in range(N_KT)]
        plan.append(row)
    return plan, strips, biases


def _build(plan, n_strips, n_biases):
    import concourse.bass as bass
    import concourse.bacc as bacc
    import concourse.tile as tile
    import concourse.mybir as mybir
    from contextlib import ExitStack

    f32 = mybir.dt.float32
    f16 = mybir.dt.float16
    Exp = mybir.ActivationFunctionType.Exp
    Ident = mybir.ActivationFunctionType.Identity

    nc = bacc.Bacc(trn_type="TRN2", target_bir_lowering=False, debug=False)
    # x pre-tiled host-side: [sb, k, 128, SB] so each (sb) load is contiguous
    xTt = nc.dram_tensor("xTt", [S // SB, D // 128, 128, SB], f16,
                         kind="ExternalInput").ap()
    w_qk = nc.dram_tensor("w_qk", [D, 2 * GD], f16, kind="ExternalInput").ap()
    b_qk = nc.dram_tensor("b_qk", [128, 2 * GD // 128], f32,
                          kind="ExternalInput").ap()
    w_v = nc.dram_tensor("w_v", [D, GD], f16, kind="ExternalInput").ap()
    wo_T = nc.dram_tensor("wo_T", [GD, D], f16, kind="ExternalInput").ap()
    maskm = nc.dram_tensor("maskm", [max(n_strips, 1), KTL, STRW], f16,
                           kind="ExternalInput").ap()
    maskb = nc.dram_tensor("maskb", [max(n_biases, 1), KTL, QT], f32,
                           kind="ExternalInput").ap()
    # output tiled [m, qi, 128, QT] so each store is contiguous
    outTt = nc.dram_tensor("outTt", [D // 128, N_QT, 128, QT], f16,
                           kind="ExternalOutput").ap()

    ND = D // 128       # 8 contraction chunks
    NM = 2 * GD // 128  # 8 QK feature chunks (0-3 = Q.T, 4-7 = K.T)
    NK3 = GD // 128     # 4 output-projection contraction chunks

    with tile.TileContext(nc) as tc, ExitStack() as ctx:
        singles = ctx.enter_context(tc.tile_pool(name="singles", bufs=1))
        qkt_pool = ctx.enter_context(tc.tile_pool(name="qkt", bufs=1))
        v_pool = ctx.enter_context(tc.tile_pool(name="vp", bufs=1))
        otq_pool = ctx.enter_context(tc.tile_pool(name="otq", bufs=2))

        qall = qkt_pool.tile([128, NM, S], f16, tag="qkt", name="qall")
        v_sb = [v_pool.tile([128, HPC, 128], f16, tag=f"v{t}", name=f"v{t}")
                for t in range(N_KT)]
        # trigger the gpsimd library load and the scalar activation-table
        # load now, while those engines are idle — the first
        # partition_broadcast otherwise pays ~7us and the first activation
        # ~1.3us mid-pipeline.
        gs_warm = singles.tile([2, 8], f32)
        nc.vector.memset(gs_warm[0:1, :], 0.0)
        nc.gpsimd.partition_broadcast(gs_warm[:], gs_warm[0:1, :], channels=2)
        nc.scalar.activation(out=gs_warm[0:1, :], in_=gs_warm[0:1, :],
                             func=Exp, scale=1.0)
        bqk_t = singles.tile([128, NM], f32)
        mm_t = [singles.tile([KTL, STRW], f16, tag=f"mm{i}", name=f"mm{i}")
                for i in range(n_strips)]
        mb_t = [singles.tile([KTL, QT], f32, tag=f"mb{i}", name=f"mb{i}")
                for i in range(n_biases)]

        # ============ phase 1: QKV projection (single x pass) ============
        # wv and the last x block live in `singles` (not the phase-1 pools)
        # because the last V block is deferred into the attention stream.
        wv_t = singles.tile([128, ND, GD], f16, tag="wv", name="wv")
        xs_last = singles.tile([128, ND, SB], f16, tag="xlast", name="xs_last")
        xs_last2 = singles.tile([128, ND, SB], f16, tag="xlast2", name="xs_last2")
        LSB = S // SB - 1
        with tc.tile_pool(name="p1w", bufs=1) as p1w, \
             tc.tile_pool(name="p1x", bufs=3) as p1x, \
             tc.tile_pool(name="p1ps", bufs=8, space="PSUM") as p1ps:
            wqk_t = p1w.tile([128, ND, 2 * GD], f16, tag="wqk", name="wqk")
            w_qk_r = w_qk.rearrange("(c p) m -> p c m", p=128)
            w_v_r = w_v.rearrange("(c p) m -> p c m", p=128)

            def xs_alloc_dma(sb):
                t = p1x.tile([128, ND, SB], f16, tag="x", name=f"xs{sb}")
                nc.sync.dma_start(
                    out=t[:], in_=xTt[sb].rearrange("k p s -> p k s"))
                return t

            # DMA priority: the DMA engines round-robin across all queued
            # transfers, so the chunks the first matmuls need (x, wv for the
            # V-first block) are issued as small pieces; bulk follows.
            nc.sync.dma_start(out=bqk_t, in_=b_qk)
            xs_sb = {}
            xs_sb[0] = p1x.tile([128, ND, SB], f16, tag="x", name="xs0")
            for k in range(ND):
                nc.sync.dma_start(
                    out=xs_sb[0][:, k, :],
                    in_=xTt[0, k].rearrange("p s -> p s"))
                nc.sync.dma_start(out=wv_t[:, k, :], in_=w_v_r[:, k, :])
            for k in range(ND):
                nc.sync.dma_start(out=wqk_t[:, k, :], in_=w_qk_r[:, k, :])
            xs_sb[1] = xs_alloc_dma(1)
            nc.sync.dma_start(out=xs_last2[:],
                              in_=xTt[LSB - 1].rearrange("k p s -> p k s"))
            xs_sb[2] = xs_last2
            for i in range(n_strips):
                nc.sync.dma_start(out=mm_t[i], in_=maskm[i])
            for i in range(n_biases):
                nc.sync.dma_start(out=mb_t[i], in_=maskb[i])
            for sb in range(S // SB):
                xs = xs_sb.pop(sb)
                # V first: its PSUM tiles come from banks freed by the
                # previous block's bias-activations, so QK-after-V keeps the
                # PE independent of the scalar engine's progress. The last
                # two blocks' V is deferred into the attention stream.
                if sb < LSB - 1:
                    for tt in range(SB // 128):
                        t = sb * (SB // 128) + tt
                        ps = p1ps.tile([128, GD], f32, tag="p1", name="ps_v")
                        for k in range(ND):
                            nc.tensor.matmul(
                                ps[:], xs[:, k, 128 * tt:128 * (tt + 1)],
                                wv_t[:, k, :],
                                start=(k == 0), stop=(k == ND - 1))
                        nc.vector.tensor_copy(
                            out=v_sb[t][:, :, DK:2 * DK],
                            in_=ps[:].rearrange("p (h d) -> p h d", h=HPC))
                        nc.vector.memset(v_sb[t][:, :, 0:1], 1.0)
                        nc.vector.memset(v_sb[t][:, :, 1:DK], 0.0)
                pss = [p1ps.tile([128, SB], f32, tag="p1", name=f"ps{m}")
                       for m in range(NM)]
                # last block: m-outer so each pss[m] finishes (and its
                # bias-activation can run) before the block's matmuls end,
                # K of hp0 first — the first attention stream needs it.
                m_order = ([4, 0, 5, 1, 6, 2, 7, 3] if sb == LSB
                           else list(range(NM)))
                if sb == LSB:
                    for m in m_order:
                        for k in range(ND):
                            nc.tensor.matmul(
                                pss[m][:], wqk_t[:, k, 128 * m:128 * (m + 1)],
                                xs[:, k, :],
                                start=(k == 0), stop=(k == ND - 1))
                        nc.scalar.activation(
                            out=qall[:, m, SB * sb:SB * (sb + 1)], in_=pss[m][:],
                            func=Ident, bias=bqk_t[:, m:m + 1], scale=1.0)
                else:
                    for k in range(ND):
                        for m in range(NM):
                            nc.tensor.matmul(
                                pss[m][:], wqk_t[:, k, 128 * m:128 * (m + 1)],
                                xs[:, k, :],
                                start=(k == 0), stop=(k == ND - 1))
                    for m in m_order:
                        nc.scalar.activation(
                            out=qall[:, m, SB * sb:SB * (sb + 1)], in_=pss[m][:],
                            func=Ident, bias=bqk_t[:, m:m + 1], scale=1.0)
                if sb + 3 < S // SB:
                    if sb + 3 == LSB:
                        nc.sync.dma_start(
                            out=xs_last[:],
                            in_=xTt[LSB].rearrange("k p s -> p k s"))
                        xs_sb[LSB] = xs_last
                    else:
                        xs_sb[sb + 3] = xs_alloc_dma(sb + 3)

        # wo load after phase-1 weights are gone (SBUF headroom)
        wo_t2 = singles.tile([128, NK3, D], f16, tag="wo2", name="wo2")
        nc.sync.dma_start(out=wo_t2[:],
                          in_=wo_T.rearrange("(c p) m -> p c m", p=128))

        # ===== phase 2+3: attention + output projection, software-pipelined =====
        st_pool = ctx.enter_context(tc.tile_pool(name="st", bufs=2, space="PSUM"))
        ot_pool = ctx.enter_context(tc.tile_pool(name="ot", bufs=2, space="PSUM"))
        p3ps = ctx.enter_context(tc.tile_pool(name="p3ps", bufs=2, space="PSUM"))
        pt_pool = ctx.enter_context(tc.tile_pool(name="pt", bufs=6))
        rr_pool = ctx.enter_context(tc.tile_pool(name="rr", bufs=3))
        p3o = ctx.enter_context(tc.tile_pool(name="p3o", bufs=4))

        from collections import deque
        pending_norm = []      # flushed fully at each hp-stream start (ki==0)
        pending_mm = deque()   # out-proj m-chunks, sprinkled one per ki

        def make_normalize(hp, ot_ps, otq):
            def fn():
                rbs = []
                for h in range(2):
                    r_row = rr_pool.tile([1, QT], f32, tag="rrow", name="r_row")
                    nc.vector.reciprocal_approx_fast(out=r_row[:],
                                                     in_=ot_ps[h][0:1, :])
                    rb_sb = rr_pool.tile([DK, QT], f32, tag="rbsb", name="rb_sb")
                    nc.gpsimd.partition_broadcast(rb_sb[:], r_row[:], channels=DK)
                    rbs.append(rb_sb)
                for h in range(2):
                    nc.vector.tensor_mul(otq[hp][64 * h:64 * h + 64, :],
                                         ot_ps[h][DK:2 * DK, :], rbs[h][:])
            return fn

        drain_ctx = {"scalar_copies": False}

        def make_outproj_parts(qi, otq):
            # one closure per (m, k) single matmul so the out-projection can
            # be sprinkled into the attention stream at fine grain
            state = {}
            parts = []
            for m in range(D // 128):
                for k in range(NK3):
                    def fn(m=m, k=k):
                        if k == 0:
                            state[m] = p3ps.tile([128, QT], f32, tag="p3",
                                                 name="ps_o")
                        ps = state[m]
                        nc.tensor.matmul(
                            ps[:], wo_t2[:, k, 128 * m:128 * (m + 1)], otq[k][:],
                            start=(k == 0), stop=(k == NK3 - 1))
                        if k == NK3 - 1:
                            ob = p3o.tile([128, QT], f16, tag="ob", name="ob")
                            # in the endgame the scalar engine is idle while
                            # vector runs the normalize chains: split copies
                            if drain_ctx["scalar_copies"] and m % 2:
                                nc.scalar.copy(out=ob[:], in_=ps[:])
                            else:
                                nc.vector.tensor_copy(out=ob[:], in_=ps[:])
                            nc.sync.dma_start(out=outTt[m, qi], in_=ob[:])
                            del state[m]
                    parts.append(fn)
            return parts

        def emit_pv(e):
            pt, q0, kt, ki, ops, n, ehp = e
            for h in range(2):
                nc.tensor.matmul(
                    ops[h][:, q0:QT],
                    v_sb[kt][:, 2 * ehp + h, :],
                    pt[:, h, q0:QT],
                    start=(ki == 0), stop=(ki == n - 1))

        # deferred V blocks (last two x blocks): run as filler inside the
        # first attention stream. All 8 closures must flush within that
        # stream — a v_sb writer emitted after its reading PV would race
        # (Tile dependencies follow program order).
        def make_vdef(xs_tile, sbi, tt):
            def fn():
                t = sbi * (SB // 128) + tt
                ps = p3ps.tile([128, GD], f32, tag="p3", name="ps_vd")
                for k in range(ND):
                    nc.tensor.matmul(
                        ps[:], xs_tile[:, k, 128 * tt:128 * (tt + 1)],
                        wv_t[:, k, :],
                        start=(k == 0), stop=(k == ND - 1))
                nc.vector.tensor_copy(
                    out=v_sb[t][:, :, DK:2 * DK],
                    in_=ps[:].rearrange("p (h d) -> p h d", h=HPC))
                nc.vector.memset(v_sb[t][:, :, 0:1], 1.0)
                nc.vector.memset(v_sb[t][:, :, 1:DK], 0.0)
            return fn

        for sbi, xs_tile in ((LSB - 1, xs_last2), (LSB, xs_last)):
            for tt in range(SB // 128):
                pending_mm.append(make_vdef(xs_tile, sbi, tt))

        # densest q-tiles first: long matmul streams at phase-2 start keep
        # the PE busy enough that HAM stays unthrottled. Each stream's final
        # PV is carried into the next stream so tails never stall the PE.
        qi_order = sorted(range(N_QT), key=lambda q: -len(plan[q]))
        carry = None
        for qi_idx, qi in enumerate(qi_order):
            if qi_idx == len(qi_order) - 1:
                drain_ctx["scalar_copies"] = True
            otq = [otq_pool.tile([128, QT], f16, tag=f"otq{m}", name=f"otq{m}")
                   for m in range(NK3)]
            kts = plan[qi]
            nkt = len(kts)
            for hp in range(HPC // 2):
                ot_ps = [ot_pool.tile([128, QT], f32, tag="ot", name="ot_ps")
                         for _ in range(2)]
                prev = None
                for ki, (kt, kind, a, bopt) in enumerate(kts):
                    q0 = a if kind == 1 else 0
                    st = st_pool.tile([128, 2, QT], f32, tag="st", name="st")
                    for h in range(2):
                        lo, hi = 64 * h, 64 * h + 64
                        nc.tensor.matmul(
                            st[:, h, q0:QT],
                            qall[lo:hi, 4 + hp, KTL * kt:KTL * (kt + 1)],
                            qall[lo:hi, hp, QT * qi + q0:QT * (qi + 1)],
                            start=True, stop=True, tile_position=(64 * h, 0))
                    if kind == 2:
                        for h in range(2):
                            nc.vector.tensor_add(st[:, h, :], st[:, h, :], mb_t[a][:])
                    pt = pt_pool.tile([128, 2, QT], f16, tag="pt", name="pt")
                    nc.scalar.activation(out=pt[:, :, q0:QT], in_=st[:, :, q0:QT],
                                         func=Exp, scale=float(SCALE))
                    if kind == 1:
                        si, w = bopt
                        for h in range(2):
                            nc.vector.tensor_mul(pt[:, h, q0:q0 + w],
                                                 pt[:, h, q0:q0 + w],
                                                 mm_t[si][:, 0:w])
                    if ki == 0:
                        if carry is not None:
                            emit_pv(carry)
                            carry = None
                        for fn in pending_norm:
                            fn()
                        pending_norm.clear()
                        if pending_mm:
                            pending_mm.popleft()()
                    else:
                        quota = 1 + (1 if qi_idx >= 2 else 0) \
                            + (1 if len(pending_mm) > 24 else 0)
                        for _ in range(min(quota, len(pending_mm))):
                            pending_mm.popleft()()
                    if prev is not None:
                        emit_pv(prev)
                    prev = (pt, q0, kt, ki, ot_ps, nkt, hp)
                carry = prev
                pending_norm.append(make_normalize(hp, ot_ps, otq))
            pending_mm.extend(make_outproj_parts(qi, otq))
        if carry is not None:
            emit_pv(carry)
        for fn in pending_norm:
            fn()
        pending_norm.clear()
        while pending_mm:
            pending_mm.popleft()()
    nc.compile()
    return nc


def kernel(encodings_for_qkv, mask, w_qkv, b_qkv, w_o):
    global last_results
    from concourse.bass_utils import run_bass_kernel_spmd

    x = np.ascontiguousarray(np.asarray(encodings_for_qkv, dtype=np.float32))
    mask2d = np.asarray(mask).reshape(S, S).astype(bool)
    w_qkv = np.asarray(w_qkv, dtype=np.float32)
    b_qkv = np.asarray(b_qkv, dtype=np.float32)
    w_o = np.asarray(w_o, dtype=np.float32)

    plan, strips, biases = _classify_mask(mask2d)
    key = repr([[e[:3] + ((e[3][0], e[3][1]) if e[3] else None,) for e in row]
                for row in plan]) + repr(sorted(
                    (k, v) for k, v in os.environ.items() if k.startswith("KERNEL_")))
    if key not in _cache:
        _cache[key] = _build(plan, len(strips), len(biases))
    nc = _cache[key]

    maskm = (np.stack(strips) if strips
             else np.zeros((1, KTL, STRW), dtype=np.float32))
    maskb = (np.stack(biases) if biases
             else np.zeros((1, KTL, QT), dtype=np.float32))
    wT = np.ascontiguousarray(w_qkv.T)        # [D, 3D]
    woT_full = w_o.T                          # [D(in), D(out)]

    in_maps = []
    for c in range(8):
        b, g = divmod(c, 2)
        cols = slice(GD * g, GD * (g + 1))
        w_qk_g = np.ascontiguousarray(
            np.concatenate([wT[:, 0 * D:][:, cols], wT[:, 1 * D:][:, cols]], axis=1))
        b_qk_g = np.ascontiguousarray(
            np.concatenate([b_qkv[0 * D:1 * D][cols], b_qkv[1 * D:2 * D][cols]])
            .reshape(2 * GD // 128, 128).T)
        w_v_g = np.ascontiguousarray(wT[:, 2 * D:][:, cols])
        wo_T_g = np.ascontiguousarray(woT_full[cols, :])
        # x[b].T tiled to [sb, k, 128, SB] so device loads are contiguous
        xtt = (x[b].T.reshape(D // 128, 128, S // SB, SB)
               .transpose(2, 0, 1, 3))
        in_maps.append({
            "xTt": np.ascontiguousarray(xtt).astype(np.float16),
            "w_qk": w_qk_g.astype(np.float16), "b_qk": b_qk_g,
            "w_v": w_v_g.astype(np.float16),
            "wo_T": wo_T_g.astype(np.float16),
            "maskm": maskm.astype(np.float16), "maskb": maskb,
        })

    trace = bool(int(os.environ.get("KERNEL_PROFILE", "0")))
    res = run_bass_kernel_spmd(nc, in_maps, core_ids=list(range(8)),
                               trace=trace,
                               trace_cores=list(range(8)) if trace else None)
    last_results = res

    out = np.empty((B, S, D), dtype=np.float32)
    for b in range(B):
        # outTt [m, qi, 128, QT] -> outT [D, S]
        acc = (res.results[2 * b]["outTt"].astype(np.float32)
               + res.results[2 * b + 1]["outTt"].astype(np.float32))
        outT_full = acc.transpose(0, 2, 1, 3).reshape(D, S)
        out[b] = outT_full.T
    # V-bias epilogue: softmax rows sum to 1, so the V bias contributes a
    # constant (b_v @ w_o.T) to every sequence position.
    out += (b_qkv[2 * D:] @ woT_full).reshape(1, 1, D)
    return out


# revision 56
# speedup vs baseline: 1.1865x; 1.1865x over previous
"""Multi-head attention (B=4, S=2048, D=1024, H=16) on 8 TRN2 NeuronCores.

Sharding: core c -> (batch b = c//2, head-group g = c%2 of 8 heads).
Data parallel over batch, tensor parallel over heads; each core computes
its group's QKV projection slices, causal attention for its 8 heads, and
the partial output projection. Host sums the two per-batch partials
(the tensor-parallel unshard) and adds the V-bias epilogue.

On-device layout is "features on partitions": x, Q, K arrive/stay
transposed [feat, seq]; attention scores are computed directly in
transposed form S.T[k, q] so the exp'd probabilities feed the PV matmul
without any on-chip transpose. The softmax denominator rides the PV
matmul as an appended ones-column of V; normalization is a fast
reciprocal + K=1 broadcast matmul + DVE multiply. Causal masking uses
suffix-restricted tiles plus a small multiplicative 0/1 strip on the
exp'd probabilities; non-staircase masks fall back to additive -1e9
biases on the scores.

The attention inner loop is software-pipelined to keep the PE array
busy (and HAM unthrottled): PV matmuls lag the score matmuls by one
k-tile, and the per-head-pair normalization + per-q-tile output
projection are deferred into the next head-pair's score stream.
"""

import os
import numpy as np

B, S, D, H = 4, 2048, 1024, 16
DK = D // H          # 64
HPC = H // 2         # heads per core = 8
GD = HPC * DK        # group feature width = 512
QT = 512             # q-tile width (free dim of S.T chunks)
KTL = 128            # k-tile length (partition dim of S.T chunks)
N_QT = S // QT       # 4
N_KT = S // KTL      # 16
SB = 512             # phase-1 seq block
STRW = 128           # width of multiplicative mask strips
NEG = np.float32(-1e9)
SCALE = 1.0 / np.sqrt(np.float32(DK))

_cache = {}
last_results = None


def _classify_mask(mask2d):
    """Classify each (q-tile, k-tile) block of the [S,S] bool mask.

    Returns (plan, strips, biases):
      plan[qi] = list over valid kt of (kt, kind, a, b):
        kind 0 = clean (no masking)
        kind 1 = staircase: a = q0 (suffix start), b = (strip_idx, strip_w)
        kind 2 = general:   a = bias_idx
      strips: list of [KTL, STRW] f32 0/1 multiplicative masks
      biases: list of [KTL, QT] f32 additive -1e9/0 masks
    Blocks are in S.T (k, q) layout.
    """
    kl = np.arange(KTL)[:, None]
    ql = np.arange(QT)[None, :]
    plan = []
    strips, strip_keys = [], {}
    biases, bias_keys = [], {}
    for qi in range(N_QT):
        row = []
        for kt in range(N_KT):
            blk = mask2d[qi * QT:(qi + 1) * QT, kt * KTL:(kt + 1) * KTL].T
            if blk.all():
                continue
            if not blk.any():
                row.append((kt, 0, 0, None))
                continue
            dj = kt * KTL - qi * QT
            stair = (0 <= dj <= QT - KTL and np.array_equal(blk, kl + dj > ql)
                     and not os.environ.get("KERNEL_NO_STAIR"))
            if stair and (dj == 0 or row):
                # masked region spans exactly q in [dj, dj+KTL)
                w = min(dj + KTL, QT) - dj
                pat = (~blk[:, dj:dj + w]).astype(np.float32)
                key = (w, pat.tobytes())
                if key not in strip_keys:
                    strip_keys[key] = len(strips)
                    p = np.zeros((KTL, STRW), np.float32)
                    p[:, :w] = pat
                    strips.append(p)
                row.append((kt, 1, dj, (strip_keys[key], w)))
            else:
                bias = np.where(blk, NEG, np.float32(0.0))
                key = bias.tobytes()
                if key not in bias_keys:
                    bias_keys[key] = len(biases)
                    biases.append(bias)
                row.append((kt, 2, bias_keys[key], None))
        if not row:
            # fully-masked q-row: include everything with full bias so the
            # softmax matches the reference's uniform distribution.
            bias = np.full((KTL, QT), NEG, np.float32)
            key = bias.tobytes()
            if key not in bias_keys:
                bias_keys[key] = len(biases)
                biases.append(bias)
            row = [(kt, 2, bias_keys[key], None) for kt in range(N_KT)]
        plan.append(row)
    return plan, strips, biases


def _build(plan, n_strips, n_biases):
    import concourse.bass as bass
    import concourse.bacc as bacc
    import concourse.tile as tile
    import concourse.mybir as mybir
    from contextlib import ExitStack

    f32 = mybir.dt.float32
    f16 = mybir.dt.float16
    Exp = mybir.ActivationFunctionType.Exp
    Ident = mybir.ActivationFunctionType.Identity

    nc = bacc.Bacc(trn_type="TRN2", target_bir_lowering=False, debug=False)
    # x pre-tiled host-side: [sb, k, 128, SB] so each (sb) load is contiguous
    xTt = nc.dram_tensor("xTt", [S // SB, D // 128, 128, SB], f16,
                         kind="ExternalInput").ap()
    w_qk = nc.dram_tensor("w_qk", [D, 2 * GD], f16, kind="ExternalInput").ap()
    b_qk = nc.dram_tensor("b_qk", [128, 2 * GD // 128], f32,
                          kind="ExternalInput").ap()
    w_v = nc.dram_tensor("w_v", [D, GD], f16, kind="ExternalInput").ap()
    wo_T = nc.dram_tensor("wo_T", [GD, D], f16, kind="ExternalInput").ap()
    maskm = nc.dram_tensor("maskm", [max(n_strips, 1), KTL, STRW], f16,
                           kind="ExternalInput").ap()
    maskb = nc.dram_tensor("maskb", [max(n_biases, 1), KTL, QT], f32,
                           kind="ExternalInput").ap()
    # output tiled [m, qi, 128, QT] so each store is contiguous
    outTt = nc.dram_tensor("outTt", [D // 128, N_QT, 128, QT], f16,
                           kind="ExternalOutput").ap()

    ND = D // 128       # 8 contraction chunks
    NM = 2 * GD // 128  # 8 QK feature chunks (0-3 = Q.T, 4-7 = K.T)
    NK3 = GD // 128     # 4 output-projection contraction chunks

    with tile.TileContext(nc) as tc, ExitStack() as ctx:
        singles = ctx.enter_context(tc.tile_pool(name="singles", bufs=1))
        qkt_pool = ctx.enter_context(tc.tile_pool(name="qkt", bufs=1))
        v_pool = ctx.enter_context(tc.tile_pool(name="vp", bufs=1))
        otq_pool = ctx.enter_context(tc.tile_pool(name="otq", bufs=2))

        qkt = [qkt_pool.tile([128, S], f16, tag=f"qkt{m}", name=f"qkt{m}")
               for m in range(NM)]
        v_sb = [v_pool.tile([128, HPC, 128], f16, tag=f"v{t}", name=f"v{t}")
                for t in range(N_KT)]
        # trigger the gpsimd library load and the scalar activation-table
        # load now, while those engines are idle — the first
        # partition_broadcast otherwise pays ~7us and the first activation
        # ~1.3us mid-pipeline.
        gs_warm = singles.tile([2, 8], f32)
        nc.vector.memset(gs_warm[0:1, :], 0.0)
        nc.gpsimd.partition_broadcast(gs_warm[:], gs_warm[0:1, :], channels=2)
        nc.scalar.activation(out=gs_warm[0:1, :], in_=gs_warm[0:1, :],
                             func=Exp, scale=1.0)
        bqk_t = singles.tile([128, NM], f32)
        mm_t = [singles.tile([KTL, STRW], f16, tag=f"mm{i}", name=f"mm{i}")
                for i in range(n_strips)]
        mb_t = [singles.tile([KTL, QT], f32, tag=f"mb{i}", name=f"mb{i}")
                for i in range(n_biases)]

        # ============ phase 1: QKV projection (single x pass) ============
        # wv and the last x block live in `singles` (not the phase-1 pools)
        # because the last V block is deferred into the attention stream.
        wv_t = singles.tile([128, ND, GD], f16, tag="wv", name="wv")
        xs_last = singles.tile([128, ND, SB], f16, tag="xlast", name="xs_last")
        xs_last2 = singles.tile([128, ND, SB], f16, tag="xlast2", name="xs_last2")
        LSB = S // SB - 1
        with tc.tile_pool(name="p1w", bufs=1) as p1w, \
             tc.tile_pool(name="p1x", bufs=3) as p1x, \
             tc.tile_pool(name="p1ps", bufs=8, space="PSUM") as p1ps:
            wqk_t = p1w.tile([128, ND, 2 * GD], f16, tag="wqk", name="wqk")
            w_qk_r = w_qk.rearrange("(c p) m -> p c m", p=128)
            w_v_r = w_v.rearrange("(c p) m -> p c m", p=128)

            def xs_alloc_dma(sb):
                t = p1x.tile([128, ND, SB], f16, tag="x", name=f"xs{sb}")
                nc.sync.dma_start(
                    out=t[:], in_=xTt[sb].rearrange("k p s -> p k s"))
                return t

            # DMA priority: the DMA engines round-robin across all queued
            # transfers, so the chunks the first matmuls need (x, wv for the
            # V-first block) are issued as small pieces; bulk follows.
            nc.sync.dma_start(out=bqk_t, in_=b_qk)
            xs_sb = {}
            xs_sb[0] = p1x.tile([128, ND, SB], f16, tag="x", name="xs0")
            for k in range(ND):
                nc.sync.dma_start(
                    out=xs_sb[0][:, k, :],
                    in_=xTt[0, k].rearrange("p s -> p s"))
                nc.sync.dma_start(out=wv_t[:, k, :], in_=w_v_r[:, k, :])
            for k in range(ND):
                nc.sync.dma_start(out=wqk_t[:, k, :], in_=w_qk_r[:, k, :])
            xs_sb[1] = xs_alloc_dma(1)
            nc.sync.dma_start(out=xs_last2[:],
                              in_=xTt[LSB - 1].rearrange("k p s -> p k s"))
            xs_sb[2] = xs_last2
            for i in range(n_strips):
                nc.sync.dma_start(out=mm_t[i], in_=maskm[i])
            for i in range(n_biases):
                nc.sync.dma_start(out=mb_t[i], in_=maskb[i])
            for sb in range(S // SB):
                xs = xs_sb.pop(sb)
                # V first: its PSUM tiles come from banks freed by the
                # previous block's bias-activations, so QK-after-V keeps the
                # PE independent of the scalar engine's progress. The last
                # two blocks' V is deferred into the attention stream.
                if sb < LSB - 1:
                    for tt in range(SB // 128):
                        t = sb * (SB // 128) + tt
                        ps = p1ps.tile([128, GD], f32, tag="p1", name="ps_v")
                        for k in range(ND):
                            nc.tensor.matmul(
                                ps[:], xs[:, k, 128 * tt:128 * (tt + 1)],
                                wv_t[:, k, :],
                                start=(k == 0), stop=(k == ND - 1))
                        nc.vector.tensor_copy(
                            out=v_sb[t][:, :, DK:2 * DK],
                            in_=ps[:].rearrange("p (h d) -> p h d", h=HPC))
                        nc.vector.memset(v_sb[t][:, :, 0:1], 1.0)
                        nc.vector.memset(v_sb[t][:, :, 1:DK], 0.0)
                pss = [p1ps.tile([128, SB], f32, tag="p1", name=f"ps{m}")
                       for m in range(NM)]
                # last block: m-outer so each pss[m] finishes (and its
                # bias-activation can run) before the block's matmuls end,
                # K of hp0 first — the first attention stream needs it.
                m_order = ([4, 0, 5, 1, 6, 2, 7, 3] if sb == LSB
                           else list(range(NM)))
                if sb == LSB:
                    for m in m_order:
                        for k in range(ND):
                            nc.tensor.matmul(
                                pss[m][:], wqk_t[:, k, 128 * m:128 * (m + 1)],
                                xs[:, k, :],
                                start=(k == 0), stop=(k == ND - 1))
                        nc.scalar.activation(
                            out=qkt[m][:, SB * sb:SB * (sb + 1)], in_=pss[m][:],
                            func=Ident, bias=bqk_t[:, m:m + 1], scale=1.0)
                else:
                    for k in range(ND):
                        for m in range(NM):
                            nc.tensor.matmul(
                                pss[m][:], wqk_t[:, k, 128 * m:128 * (m + 1)],
                                xs[:, k, :],
                                start=(k == 0), stop=(k == ND - 1))
                    for m in m_order:
                        nc.scalar.activation(
                            out=qkt[m][:, SB * sb:SB * (sb + 1)], in_=pss[m][:],
                            func=Ident, bias=bqk_t[:, m:m + 1], scale=1.0)
                if sb + 3 < S // SB:
                    if sb + 3 == LSB:
                        nc.sync.dma_start(
                            out=xs_last[:],
                            in_=xTt[LSB].rearrange("k p s -> p k s"))
                        xs_sb[LSB] = xs_last
                    else:
                        xs_sb[sb + 3] = xs_alloc_dma(sb + 3)

        # wo load after phase-1 weights are gone (SBUF headroom)
        wo_t2 = singles.tile([128, NK3, D], f16, tag="wo2", name="wo2")
        nc.sync.dma_start(out=wo_t2[:],
                          in_=wo_T.rearrange("(c p) m -> p c m", p=128))

        # ===== phase 2+3: attention + output projection, software-pipelined =====
        st_pool = ctx.enter_context(tc.tile_pool(name="st", bufs=2, space="PSUM"))
        ot_pool = ctx.enter_context(tc.tile_pool(name="ot", bufs=2, space="PSUM"))
        p3ps = ctx.enter_context(tc.tile_pool(name="p3ps", bufs=2, space="PSUM"))
        pt_pool = ctx.enter_context(tc.tile_pool(name="pt", bufs=6))
        rr_pool = ctx.enter_context(tc.tile_pool(name="rr", bufs=3))
        p3o = ctx.enter_context(tc.tile_pool(name="p3o", bufs=4))

        from collections import deque
        pending_norm = []      # flushed fully at each hp-stream start (ki==0)
        pending_mm = deque()   # out-proj m-chunks, sprinkled one per ki

        def make_normalize(hp, ot_ps, otq):
            def fn():
                rbs = []
                for h in range(2):
                    r_row = rr_pool.tile([1, QT], f32, tag="rrow", name="r_row")
                    nc.vector.reciprocal_approx_fast(out=r_row[:],
                                                     in_=ot_ps[h][0:1, :])
                    rb_sb = rr_pool.tile([DK, QT], f32, tag="rbsb", name="rb_sb")
                    nc.gpsimd.partition_broadcast(rb_sb[:], r_row[:], channels=DK)
                    rbs.append(rb_sb)
                for h in range(2):
                    nc.vector.tensor_mul(otq[hp][64 * h:64 * h + 64, :],
                                         ot_ps[h][DK:2 * DK, :], rbs[h][:])
            return fn

        drain_ctx = {"scalar_copies": False}

        def make_outproj_parts(qi, otq):
            # one closure per (m, k) single matmul so the out-projection can
            # be sprinkled into the attention stream at fine grain
            state = {}
            parts = []
            for m in range(D // 128):
                for k in range(NK3):
                    def fn(m=m, k=k):
                        if k == 0:
                            state[m] = p3ps.tile([128, QT], f32, tag="p3",
                                                 name="ps_o")
                        ps = state[m]
                        nc.tensor.matmul(
                            ps[:], wo_t2[:, k, 128 * m:128 * (m + 1)], otq[k][:],
                            start=(k == 0), stop=(k == NK3 - 1))
                        if k == NK3 - 1:
                            ob = p3o.tile([128, QT], f16, tag="ob", name="ob")
                            # in the endgame the scalar engine is idle while
                            # vector runs the normalize chains: split copies
                            if drain_ctx["scalar_copies"] and m % 2:
                                nc.scalar.copy(out=ob[:], in_=ps[:])
                            else:
                                nc.vector.tensor_copy(out=ob[:], in_=ps[:])
                            nc.sync.dma_start(out=outTt[m, qi], in_=ob[:])
                            del state[m]
                    parts.append(fn)
            return parts

        def emit_pv(e):
            pt, q0, kt, ki, ops, n, ehp = e
            for h in range(2):
                nc.tensor.matmul(
                    ops[h][:, q0:QT],
                    v_sb[kt][:, 2 * ehp + h, :],
                    pt[:, h, q0:QT],
                    start=(ki == 0), stop=(ki == n - 1))

        # deferred V blocks (last two x blocks): run as filler inside the
        # first attention stream. All 8 closures must flush within that
        # stream — a v_sb writer emitted after its reading PV would race
        # (Tile dependencies follow program order).
        def make_vdef(xs_tile, sbi, tt):
            def fn():
                t = sbi * (SB // 128) + tt
                ps = p3ps.tile([128, GD], f32, tag="p3", name="ps_vd")
                for k in range(ND):
                    nc.tensor.matmul(
                        ps[:], xs_tile[:, k, 128 * tt:128 * (tt + 1)],
                        wv_t[:, k, :],
                        start=(k == 0), stop=(k == ND - 1))
                nc.vector.tensor_copy(
                    out=v_sb[t][:, :, DK:2 * DK],
                    in_=ps[:].rearrange("p (h d) -> p h d", h=HPC))
                nc.vector.memset(v_sb[t][:, :, 0:1], 1.0)
                nc.vector.memset(v_sb[t][:, :, 1:DK], 0.0)
            return fn

        for sbi, xs_tile in ((LSB - 1, xs_last2), (LSB, xs_last)):
            for tt in range(SB // 128):
                pending_mm.append(make_vdef(xs_tile, sbi, tt))

        # densest q-tiles first: long matmul streams at phase-2 start keep
        # the PE busy enough that HAM stays unthrottled. Each stream's final
        # PV is carried into the next stream so tails never stall the PE.
        qi_order = sorted(range(N_QT), key=lambda q: -len(plan[q]))
        carry = None
        for qi_idx, qi in enumerate(qi_order):
            if qi_idx == len(qi_order) - 1:
                drain_ctx["scalar_copies"] = True
            otq = [otq_pool.tile([128, QT], f16, tag=f"otq{m}", name=f"otq{m}")
                   for m in range(NK3)]
            kts = plan[qi]
            nkt = len(kts)
            for hp in range(HPC // 2):
                ot_ps = [ot_pool.tile([128, QT], f32, tag="ot", name="ot_ps")
                         for _ in range(2)]
                prev = None
                for ki, (kt, kind, a, bopt) in enumerate(kts):
                    q0 = a if kind == 1 else 0
                    st = st_pool.tile([128, 2, QT], f32, tag="st", name="st")
                    for h in range(2):
                        lo, hi = 64 * h, 64 * h + 64
                        nc.tensor.matmul(
                            st[:, h, q0:QT],
                            qkt[4 + hp][lo:hi, KTL * kt:KTL * (kt + 1)],
                            qkt[hp][lo:hi, QT * qi + q0:QT * (qi + 1)],
                            start=True, stop=True, tile_position=(64 * h, 0))
                    if kind == 2:
                        for h in range(2):
                            nc.vector.tensor_add(st[:, h, :], st[:, h, :], mb_t[a][:])
                    pt = pt_pool.tile([128, 2, QT], f16, tag="pt", name="pt")
                    nc.scalar.activation(out=pt[:, :, q0:QT], in_=st[:, :, q0:QT],
                                         func=Exp, scale=float(SCALE))
                    if kind == 1:
                        si, w = bopt
                        for h in range(2):
                            nc.vector.tensor_mul(pt[:, h, q0:q0 + w],
                                                 pt[:, h, q0:q0 + w],
                                                 mm_t[si][:, 0:w])
                    if ki == 0:
                        if carry is not None:
                            emit_pv(carry)
                            carry = None
                        for fn in pending_norm:
                            fn()
                        pending_norm.clear()
                        if pending_mm:
                            pending_mm.popleft()()
                    else:
                        quota = 1 + (1 if qi_idx >= 2 else 0) \
                            + (1 if len(pending_mm) > 24 else 0)
                        for _ in range(min(quota, len(pending_mm))):
                            pending_mm.popleft()()
                    if prev is not None:
                        emit_pv(prev)
                    prev = (pt, q0, kt, ki, ot_ps, nkt, hp)
                carry = prev
                pending_norm.append(make_normalize(hp, ot_ps, otq))
            pending_mm.extend(make_outproj_parts(qi, otq))
        if carry is not None:
            emit_pv(carry)
        for fn in pending_norm:
            fn()
        pending_norm.clear()
        while pending_mm:
            pending_mm.popleft()()
    nc.compile()
    return nc


def kernel(encodings_for_qkv, mask, w_qkv, b_qkv, w_o):
    global last_results
    from concourse.bass_utils import run_bass_kernel_spmd

    x = np.ascontiguousarray(np.asarray(encodings_for_qkv, dtype=np.float32))
    mask2d = np.asarray(mask).reshape(S, S).astype(bool)
    w_qkv = np.asarray(w_qkv, dtype=np.float32)
    b_qkv = np.asarray(b_qkv, dtype=np.float32)
    w_o = np.asarray(w_o, dtype=np.float32)

    plan, strips, biases = _classify_mask(mask2d)
    key = repr([[e[:3] + ((e[3][0], e[3][1]) if e[3] else None,) for e in row]
                for row in plan]) + repr(sorted(
                    (k, v) for k, v in os.environ.items() if k.startswith("KERNEL_")))
    if key not in _cache:
        _cache[key] = _build(plan, len(strips), len(biases))
    nc = _cache[key]

    maskm = (np.stack(strips) if strips
             else np.zeros((1, KTL, STRW), dtype=np.float32))
    maskb = (np.stack(biases) if biases
             else np.zeros((1, KTL, QT), dtype=np.float32))
    wT = np.ascontiguousarray(w_qkv.T)        # [D, 3D]
    woT_full = w_o.T                          # [D(in), D(out)]

    in_maps = []
    for c in range(8):
        b, g = divmod(c, 2)
        cols = slice(GD * g, GD * (g + 1))
        w_qk_g = np.ascontiguousarray(
            np.concatenate([wT[:, 0 * D:][:, cols], wT[:, 1 * D:][:, cols]], axis=1))
        b_qk_g = np.ascontiguousarray(
            np.concatenate([b_qkv[0 * D:1 * D][cols], b_qkv[1 * D:2 * D][cols]])
            .reshape(2 * GD // 128, 128).T)
        w_v_g = np.ascontiguousarray(wT[:, 2 * D:][:, cols])
        wo_T_g = np.ascontiguousarray(woT_full[cols, :])
        # x[b].T tiled to [sb, k, 128, SB] so device loads are contiguous
        xtt = (x[b].T.reshape(D // 128, 128, S // SB, SB)
               .transpose(2, 0, 1, 3))
        in_maps.append({
            "xTt": np.ascontiguousarray(xtt).astype(np.float16),
            "w_qk": w_qk_g.astype(np.float16), "b_qk": b_qk_g,
            "w_v": w_v_g.astype(np.float16),
            "wo_T": wo_T_g.astype(np.float16),
            "maskm": maskm.astype(np.float16), "maskb": maskb,
        })

    trace = bool(int(os.environ.get("KERNEL_PROFILE", "0")))
    res = run_bass_kernel_spmd(nc, in_maps, core_ids=list(range(8)),
                               trace=trace,
                               trace_cores=list(range(8)) if trace else None)
    last_results = res

    out = np.empty((B, S, D), dtype=np.float32)
    for b in range(B):
        # outTt [m, qi, 128, QT] -> outT [D, S]
        acc = (res.results[2 * b]["outTt"].astype(np.float32)
               + res.results[2 * b + 1]["outTt"].astype(np.float32))
        outT_full = acc.transpose(0, 2, 1, 3).reshape(D, S)
        out[b] = outT_full.T
    # V-bias epilogue: softmax rows sum to 1, so the V bias contributes a
    # constant (b_v @ w_o.T) to every sequence position.
    out += (b_qkv[2 * D:] @ woT_full).reshape(1, 1, D)
    return out


# revision 57
# speedup vs baseline: 1.1894x; 1.0024x over previous
"""Multi-head attention (B=4, S=2048, D=1024, H=16) on 8 TRN2 NeuronCores.

Sharding: core c -> (batch b = c//2, head-group g = c%2 of 8 heads).
Data parallel over batch, tensor parallel over heads; each core computes
its group's QKV projection slices, causal attention for its 8 heads, and
the partial output projection. Host sums the two per-batch partials
(the tensor-parallel unshard) and adds the V-bias epilogue.

On-device layout is "features on partitions": x, Q, K arrive/stay
transposed [feat, seq]; attention scores are computed directly in
transposed form S.T[k, q] so the exp'd probabilities feed the PV matmul
without any on-chip transpose. The softmax denominator rides the PV
matmul as an appended ones-column of V; normalization is a fast
reciprocal + K=1 broadcast matmul + DVE multiply. Causal masking uses
suffix-restricted tiles plus a small multiplicative 0/1 strip on the
exp'd probabilities; non-staircase masks fall back to additive -1e9
biases on the scores.

The attention inner loop is software-pipelined to keep the PE array
busy (and HAM unthrottled): PV matmuls lag the score matmuls by one
k-tile, and the per-head-pair normalization + per-q-tile output
projection are deferred into the next head-pair's score stream.
"""

import os
import numpy as np

B, S, D, H = 4, 2048, 1024, 16
DK = D // H          # 64
HPC = H // 2         # heads per core = 8
GD = HPC * DK        # group feature width = 512
QT = 512             # q-tile width (free dim of S.T chunks)
KTL = 128            # k-tile length (partition dim of S.T chunks)
N_QT = S // QT       # 4
N_KT = S // KTL      # 16
SB = 512             # phase-1 seq block
STRW = 128           # width of multiplicative mask strips
NEG = np.float32(-1e9)
SCALE = 1.0 / np.sqrt(np.float32(DK))

_cache = {}
last_results = None


def _classify_mask(mask2d):
    """Classify each (q-tile, k-tile) block of the [S,S] bool mask.

    Returns (plan, strips, biases):
      plan[qi] = list over valid kt of (kt, kind, a, b):
        kind 0 = clean (no masking)
        kind 1 = staircase: a = q0 (suffix start), b = (strip_idx, strip_w)
        kind 2 = general:   a = bias_idx
      strips: list of [KTL, STRW] f32 0/1 multiplicative masks
      biases: list of [KTL, QT] f32 additive -1e9/0 masks
    Blocks are in S.T (k, q) layout.
    """
    kl = np.arange(KTL)[:, None]
    ql = np.arange(QT)[None, :]
    plan = []
    strips, strip_keys = [], {}
    biases, bias_keys = [], {}
    for qi in range(N_QT):
        row = []
        for kt in range(N_KT):
            blk = mask2d[qi * QT:(qi + 1) * QT, kt * KTL:(kt + 1) * KTL].T
            if blk.all():
                continue
            if not blk.any():
                row.append((kt, 0, 0, None))
                continue
            dj = kt * KTL - qi * QT
            stair = (0 <= dj <= QT - KTL and np.array_equal(blk, kl + dj > ql)
                     and not os.environ.get("KERNEL_NO_STAIR"))
            if stair and (dj == 0 or row):
                # masked region spans exactly q in [dj, dj+KTL)
                w = min(dj + KTL, QT) - dj
                pat = (~blk[:, dj:dj + w]).astype(np.float32)
                key = (w, pat.tobytes())
                if key not in strip_keys:
                    strip_keys[key] = len(strips)
                    p = np.zeros((KTL, STRW), np.float32)
                    p[:, :w] = pat
                    strips.append(p)
                row.append((kt, 1, dj, (strip_keys[key], w)))
            else:
                bias = np.where(blk, NEG, np.float32(0.0))
                key = bias.tobytes()
                if key not in bias_keys:
                    bias_keys[key] = len(biases)
                    biases.append(bias)
                row.append((kt, 2, bias_keys[key], None))
        if not row:
            # fully-masked q-row: include everything with full bias so the
            # softmax matches the reference's uniform distribution.
            bias = np.full((KTL, QT), NEG, np.float32)
            key = bias.tobytes()
            if key not in bias_keys:
                bias_keys[key] = len(biases)
                biases.append(bias)
            row = [(kt, 2, bias_keys[key], None) for kt in range(N_KT)]
        plan.append(row)
    return plan, strips, biases


def _build(plan, n_strips, n_biases):
    import concourse.bass as bass
    import concourse.bacc as bacc
    import concourse.tile as tile
    import concourse.mybir as mybir
    from contextlib import ExitStack

    f32 = mybir.dt.float32
    f16 = mybir.dt.float16
    Exp = mybir.ActivationFunctionType.Exp
    Ident = mybir.ActivationFunctionType.Identity

    nc = bacc.Bacc(trn_type="TRN2", target_bir_lowering=False, debug=False)
    # x pre-tiled host-side: [sb, k, 128, SB] so each (sb) load is contiguous
    xTt = nc.dram_tensor("xTt", [S // SB, D // 128, 128, SB], f16,
                         kind="ExternalInput").ap()
    w_qk = nc.dram_tensor("w_qk", [D, 2 * GD], f16, kind="ExternalInput").ap()
    b_qk = nc.dram_tensor("b_qk", [128, 2 * GD // 128], f32,
                          kind="ExternalInput").ap()
    w_v = nc.dram_tensor("w_v", [D, GD], f16, kind="ExternalInput").ap()
    wo_T = nc.dram_tensor("wo_T", [GD, D], f16, kind="ExternalInput").ap()
    maskm = nc.dram_tensor("maskm", [max(n_strips, 1), KTL, STRW], f16,
                           kind="ExternalInput").ap()
    maskb = nc.dram_tensor("maskb", [max(n_biases, 1), KTL, QT], f32,
                           kind="ExternalInput").ap()
    # output tiled [m, qi, 128, QT] so each store is contiguous
    outTt = nc.dram_tensor("outTt", [D // 128, N_QT, 128, QT], f16,
                           kind="ExternalOutput").ap()

    ND = D // 128       # 8 contraction chunks
    NM = 2 * GD // 128  # 8 QK feature chunks (0-3 = Q.T, 4-7 = K.T)
    NK3 = GD // 128     # 4 output-projection contraction chunks

    with tile.TileContext(nc) as tc, ExitStack() as ctx:
        singles = ctx.enter_context(tc.tile_pool(name="singles", bufs=1))
        qkt_pool = ctx.enter_context(tc.tile_pool(name="qkt", bufs=1))
        v_pool = ctx.enter_context(tc.tile_pool(name="vp", bufs=1))
        otq_pool = ctx.enter_context(tc.tile_pool(name="otq", bufs=2))

        qkt = [qkt_pool.tile([128, S], f16, tag=f"qkt{m}", name=f"qkt{m}")
               for m in range(NM)]
        v_sb = [v_pool.tile([128, HPC, 128], f16, tag=f"v{t}", name=f"v{t}")
                for t in range(N_KT)]
        # trigger the gpsimd library load and the scalar activation-table
        # load now, while those engines are idle — the first
        # partition_broadcast otherwise pays ~7us and the first activation
        # ~1.3us mid-pipeline.
        gs_warm = singles.tile([2, 8], f32)
        nc.vector.memset(gs_warm[0:1, :], 0.0)
        nc.gpsimd.partition_broadcast(gs_warm[:], gs_warm[0:1, :], channels=2)
        nc.scalar.activation(out=gs_warm[0:1, :], in_=gs_warm[0:1, :],
                             func=Exp, scale=1.0)
        bqk_t = singles.tile([128, NM], f32)
        mm_t = [singles.tile([KTL, STRW], f16, tag=f"mm{i}", name=f"mm{i}")
                for i in range(n_strips)]
        mb_t = [singles.tile([KTL, QT], f32, tag=f"mb{i}", name=f"mb{i}")
                for i in range(n_biases)]

        # ============ phase 1: QKV projection (single x pass) ============
        # wv and the last x block live in `singles` (not the phase-1 pools)
        # because the last V block is deferred into the attention stream.
        wv_t = singles.tile([128, ND, GD], f16, tag="wv", name="wv")
        xs_last = singles.tile([128, ND, SB], f16, tag="xlast", name="xs_last")
        xs_last2 = singles.tile([128, ND, SB], f16, tag="xlast2", name="xs_last2")
        LSB = S // SB - 1
        with tc.tile_pool(name="p1w", bufs=1) as p1w, \
             tc.tile_pool(name="p1x", bufs=3) as p1x, \
             tc.tile_pool(name="p1ps", bufs=8, space="PSUM") as p1ps:
            wqk_t = p1w.tile([128, ND, 2 * GD], f16, tag="wqk", name="wqk")
            w_qk_r = w_qk.rearrange("(c p) m -> p c m", p=128)
            w_v_r = w_v.rearrange("(c p) m -> p c m", p=128)

            def xs_alloc_dma(sb):
                t = p1x.tile([128, ND, SB], f16, tag="x", name=f"xs{sb}")
                nc.sync.dma_start(
                    out=t[:], in_=xTt[sb].rearrange("k p s -> p k s"))
                return t

            # DMA priority: the DMA engines round-robin across all queued
            # transfers, so the chunks the first matmuls need (x, wv for the
            # V-first block) are issued as small pieces; bulk follows.
            nc.sync.dma_start(out=bqk_t, in_=b_qk)
            xs_sb = {}
            xs_sb[0] = p1x.tile([128, ND, SB], f16, tag="x", name="xs0")
            for k in range(ND):
                nc.sync.dma_start(
                    out=xs_sb[0][:, k, :],
                    in_=xTt[0, k].rearrange("p s -> p s"))
                nc.sync.dma_start(out=wv_t[:, k, :], in_=w_v_r[:, k, :])
            for k in range(ND):
                nc.sync.dma_start(out=wqk_t[:, k, :], in_=w_qk_r[:, k, :])
            xs_sb[1] = xs_alloc_dma(1)
            nc.sync.dma_start(out=xs_last2[:],
                              in_=xTt[LSB - 1].rearrange("k p s -> p k s"))
            xs_sb[2] = xs_last2
            for i in range(n_strips):
                nc.sync.dma_start(out=mm_t[i], in_=maskm[i])
            for i in range(n_biases):
                nc.sync.dma_start(out=mb_t[i], in_=maskb[i])
            for sb in range(S // SB):
                xs = xs_sb.pop(sb)
                # V first: its PSUM tiles come from banks freed by the
                # previous block's bias-activations, so QK-after-V keeps the
                # PE independent of the scalar engine's progress. The last
                # two blocks' V is deferred into the attention stream.
                if sb < LSB - 1:
                    for tt in range(SB // 128):
                        t = sb * (SB // 128) + tt
                        ps = p1ps.tile([128, GD], f32, tag="p1", name="ps_v")
                        for k in range(ND):
                            nc.tensor.matmul(
                                ps[:], xs[:, k, 128 * tt:128 * (tt + 1)],
                                wv_t[:, k, :],
                                start=(k == 0), stop=(k == ND - 1))
                        nc.vector.tensor_copy(
                            out=v_sb[t][:, :, DK:2 * DK],
                            in_=ps[:].rearrange("p (h d) -> p h d", h=HPC))
                        nc.vector.memset(v_sb[t][:, :, 0:1], 1.0)
                        nc.vector.memset(v_sb[t][:, :, 1:DK], 0.0)
                pss = [p1ps.tile([128, SB], f32, tag="p1", name=f"ps{m}")
                       for m in range(NM)]
                # last block: m-outer so each pss[m] finishes (and its
                # bias-activation can run) before the block's matmuls end,
                # K of hp0 first — the first attention stream needs it.
                m_order = ([4, 0, 5, 1, 6, 2, 7, 3] if sb == LSB
                           else list(range(NM)))
                if sb == LSB:
                    for m in m_order:
                        for k in range(ND):
                            nc.tensor.matmul(
                                pss[m][:], wqk_t[:, k, 128 * m:128 * (m + 1)],
                                xs[:, k, :],
                                start=(k == 0), stop=(k == ND - 1))
                        nc.scalar.activation(
                            out=qkt[m][:, SB * sb:SB * (sb + 1)], in_=pss[m][:],
                            func=Ident, bias=bqk_t[:, m:m + 1], scale=1.0)
                else:
                    for k in range(ND):
                        for m in range(NM):
                            nc.tensor.matmul(
                                pss[m][:], wqk_t[:, k, 128 * m:128 * (m + 1)],
                                xs[:, k, :],
                                start=(k == 0), stop=(k == ND - 1))
                    for m in m_order:
                        nc.scalar.activation(
                            out=qkt[m][:, SB * sb:SB * (sb + 1)], in_=pss[m][:],
                            func=Ident, bias=bqk_t[:, m:m + 1], scale=1.0)
                if sb + 3 < S // SB:
                    if sb + 3 == LSB:
                        nc.sync.dma_start(
                            out=xs_last[:],
                            in_=xTt[LSB].rearrange("k p s -> p k s"))
                        xs_sb[LSB] = xs_last
                    else:
                        xs_sb[sb + 3] = xs_alloc_dma(sb + 3)

        # wo load after phase-1 weights are gone (SBUF headroom)
        wo_t2 = singles.tile([128, NK3, D], f16, tag="wo2", name="wo2")
        nc.sync.dma_start(out=wo_t2[:],
                          in_=wo_T.rearrange("(c p) m -> p c m", p=128))

        # ===== phase 2+3: attention + output projection, software-pipelined =====
        st_pool = ctx.enter_context(tc.tile_pool(name="st", bufs=2, space="PSUM"))
        ot_pool = ctx.enter_context(tc.tile_pool(name="ot", bufs=2, space="PSUM"))
        p3ps = ctx.enter_context(tc.tile_pool(name="p3ps", bufs=2, space="PSUM"))
        pt_pool = ctx.enter_context(tc.tile_pool(name="pt", bufs=6))
        rr_pool = ctx.enter_context(tc.tile_pool(name="rr", bufs=3))
        p3o = ctx.enter_context(tc.tile_pool(name="p3o", bufs=4))

        from collections import deque
        pending_norm = []      # flushed fully at each hp-stream start (ki==0)
        pending_mm = deque()   # out-proj m-chunks, sprinkled one per ki

        def make_normalize(hp, ot_ps, otq):
            def fn():
                rbs = []
                for h in range(2):
                    r_row = rr_pool.tile([1, QT], f32, tag="rrow", name="r_row")
                    nc.vector.reciprocal_approx_fast(out=r_row[:],
                                                     in_=ot_ps[h][0:1, :])
                    rb_sb = rr_pool.tile([DK, QT], f32, tag="rbsb", name="rb_sb")
                    nc.gpsimd.partition_broadcast(rb_sb[:], r_row[:], channels=DK)
                    rbs.append(rb_sb)
                for h in range(2):
                    nc.vector.tensor_mul(otq[hp][64 * h:64 * h + 64, :],
                                         ot_ps[h][DK:2 * DK, :], rbs[h][:])
            return fn

        drain_ctx = {"scalar_copies": False}

        def make_outproj_parts(qi, otq):
            # one closure per (m, k) single matmul so the out-projection can
            # be sprinkled into the attention stream at fine grain
            state = {}
            parts = []
            for m in range(D // 128):
                for k in range(NK3):
                    def fn(m=m, k=k):
                        if k == 0:
                            state[m] = p3ps.tile([128, QT], f32, tag="p3",
                                                 name="ps_o")
                        ps = state[m]
                        nc.tensor.matmul(
                            ps[:], wo_t2[:, k, 128 * m:128 * (m + 1)], otq[k][:],
                            start=(k == 0), stop=(k == NK3 - 1))
                        if k == NK3 - 1:
                            ob = p3o.tile([128, QT], f16, tag="ob", name="ob")
                            # in the endgame the scalar engine is idle while
                            # vector runs the normalize chains: split copies
                            if drain_ctx["scalar_copies"] and m % 2:
                                nc.scalar.copy(out=ob[:], in_=ps[:])
                            else:
                                nc.vector.tensor_copy(out=ob[:], in_=ps[:])
                            nc.sync.dma_start(out=outTt[m, qi], in_=ob[:])
                            del state[m]
                    parts.append(fn)
            return parts

        def emit_pv(e):
            pt, q0, kt, ki, ops, n, ehp = e
            for h in range(2):
                nc.tensor.matmul(
                    ops[h][:, q0:QT],
                    v_sb[kt][:, 2 * ehp + h, :],
                    pt[:, h, q0:QT],
                    start=(ki == 0), stop=(ki == n - 1))

        # deferred V blocks (last two x blocks): run as filler inside the
        # first attention stream. All 8 closures must flush within that
        # stream — a v_sb writer emitted after its reading PV would race
        # (Tile dependencies follow program order).
        def make_vdef(xs_tile, sbi, tt):
            def fn():
                t = sbi * (SB // 128) + tt
                ps = p3ps.tile([128, GD], f32, tag="p3", name="ps_vd")
                for k in range(ND):
                    nc.tensor.matmul(
                        ps[:], xs_tile[:, k, 128 * tt:128 * (tt + 1)],
                        wv_t[:, k, :],
                        start=(k == 0), stop=(k == ND - 1))
                nc.vector.tensor_copy(
                    out=v_sb[t][:, :, DK:2 * DK],
                    in_=ps[:].rearrange("p (h d) -> p h d", h=HPC))
                nc.vector.memset(v_sb[t][:, :, 0:1], 1.0)
                nc.vector.memset(v_sb[t][:, :, 1:DK], 0.0)
            return fn

        for sbi, xs_tile in ((LSB - 1, xs_last2), (LSB, xs_last)):
            for tt in range(SB // 128):
                pending_mm.append(make_vdef(xs_tile, sbi, tt))

        # densest q-tiles first: long matmul streams at phase-2 start keep
        # the PE busy enough that HAM stays unthrottled. Each stream's final
        # PV is carried into the next stream so tails never stall the PE.
        qi_order = sorted(range(N_QT), key=lambda q: -len(plan[q]))
        carry = None
        for qi_idx, qi in enumerate(qi_order):
            if qi_idx == len(qi_order) - 1:
                drain_ctx["scalar_copies"] = True
            otq = [otq_pool.tile([128, QT], f16, tag=f"otq{m}", name=f"otq{m}")
                   for m in range(NK3)]
            kts = plan[qi]
            nkt = len(kts)
            for hp in range(HPC // 2):
                ot_ps = [ot_pool.tile([128, QT], f32, tag="ot", name="ot_ps")
                         for _ in range(2)]
                prev = None
                for ki, (kt, kind, a, bopt) in enumerate(kts):
                    q0 = a if kind == 1 else 0
                    st = st_pool.tile([128, 2, QT], f32, tag="st", name="st")
                    for h in range(2):
                        lo, hi = 64 * h, 64 * h + 64
                        nc.tensor.matmul(
                            st[:, h, q0:QT],
                            qkt[4 + hp][lo:hi, KTL * kt:KTL * (kt + 1)],
                            qkt[hp][lo:hi, QT * qi + q0:QT * (qi + 1)],
                            start=True, stop=True, tile_position=(64 * h, 0))
                    if kind == 2:
                        for h in range(2):
                            nc.vector.tensor_add(st[:, h, :], st[:, h, :], mb_t[a][:])
                    pt = pt_pool.tile([128, 2, QT], f16, tag="pt", name="pt")
                    nc.scalar.activation(out=pt[:, :, q0:QT], in_=st[:, :, q0:QT],
                                         func=Exp, scale=float(SCALE))
                    if kind == 1:
                        si, w = bopt
                        for h in range(2):
                            nc.vector.tensor_mul(pt[:, h, q0:q0 + w],
                                                 pt[:, h, q0:q0 + w],
                                                 mm_t[si][:, 0:w])
                    if ki == 0:
                        if carry is not None:
                            emit_pv(carry)
                            carry = None
                        for fn in pending_norm:
                            fn()
                        pending_norm.clear()
                        if pending_mm:
                            pending_mm.popleft()()
                    else:
                        quota = 1 + (1 if qi_idx >= 1 else 0) \
                            + (1 if len(pending_mm) > 24 else 0)
                        for _ in range(min(quota, len(pending_mm))):
                            pending_mm.popleft()()
                    if prev is not None:
                        emit_pv(prev)
                    prev = (pt, q0, kt, ki, ot_ps, nkt, hp)
                carry = prev
                pending_norm.append(make_normalize(hp, ot_ps, otq))
            pending_mm.extend(make_outproj_parts(qi, otq))
        if carry is not None:
            emit_pv(carry)
        for fn in pending_norm:
            fn()
        pending_norm.clear()
        while pending_mm:
            pending_mm.popleft()()
    nc.compile()
    return nc


def kernel(encodings_for_qkv, mask, w_qkv, b_qkv, w_o):
    global last_results
    from concourse.bass_utils import run_bass_kernel_spmd

    x = np.ascontiguousarray(np.asarray(encodings_for_qkv, dtype=np.float32))
    mask2d = np.asarray(mask).reshape(S, S).astype(bool)
    w_qkv = np.asarray(w_qkv, dtype=np.float32)
    b_qkv = np.asarray(b_qkv, dtype=np.float32)
    w_o = np.asarray(w_o, dtype=np.float32)

    plan, strips, biases = _classify_mask(mask2d)
    key = repr([[e[:3] + ((e[3][0], e[3][1]) if e[3] else None,) for e in row]
                for row in plan]) + repr(sorted(
                    (k, v) for k, v in os.environ.items() if k.startswith("KERNEL_")))
    if key not in _cache:
        _cache[key] = _build(plan, len(strips), len(biases))
    nc = _cache[key]

    maskm = (np.stack(strips) if strips
             else np.zeros((1, KTL, STRW), dtype=np.float32))
    maskb = (np.stack(biases) if biases
             else np.zeros((1, KTL, QT), dtype=np.float32))
    wT = np.ascontiguousarray(w_qkv.T)        # [D, 3D]
    woT_full = w_o.T                          # [D(in), D(out)]

    in_maps = []
    for c in range(8):
        b, g = divmod(c, 2)
        cols = slice(GD * g, GD * (g + 1))
        w_qk_g = np.ascontiguousarray(
            np.concatenate([wT[:, 0 * D:][:, cols], wT[:, 1 * D:][:, cols]], axis=1))
        b_qk_g = np.ascontiguousarray(
            np.concatenate([b_qkv[0 * D:1 * D][cols], b_qkv[1 * D:2 * D][cols]])
            .reshape(2 * GD // 128, 128).T)
        w_v_g = np.ascontiguousarray(wT[:, 2 * D:][:, cols])
        wo_T_g = np.ascontiguousarray(woT_full[cols, :])
        # x[b].T tiled to [sb, k, 128, SB] so device loads are contiguous
        xtt = (x[b].T.reshape(D // 128, 128, S // SB, SB)
               .transpose(2, 0, 1, 3))
        in_maps.append({
            "xTt": np.ascontiguousarray(xtt).astype(np.float16),
            "w_qk": w_qk_g.astype(np.float16), "b_qk": b_qk_g,
            "w_v": w_v_g.astype(np.float16),
            "wo_T": wo_T_g.astype(np.float16),
            "maskm": maskm.astype(np.float16), "maskb": maskb,
        })

    trace = bool(int(os.environ.get("KERNEL_PROFILE", "0")))
    res = run_bass_kernel_spmd(nc, in_maps, core_ids=list(range(8)),
                               trace=trace,
                               trace_cores=list(range(8)) if trace else None)
    last_results = res

    out = np.empty((B, S, D), dtype=np.float32)
    for b in range(B):
        # outTt [m, qi, 128, QT] -> outT [D, S]
        acc = (res.results[2 * b]["outTt"].astype(np.float32)
               + res.results[2 * b + 1]["outTt"].astype(np.float32))
        outT_full = acc.transpose(0, 2, 1, 3).reshape(D, S)
        out[b] = outT_full.T
    # V-bias epilogue: softmax rows sum to 1, so the V bias contributes a
    # constant (b_v @ w_o.T) to every sequence position.
    out += (b_qkv[2 * D:] @ woT_full).reshape(1, 1, D)
    return out
